# revision 74
# baseline (speedup 1.0000x reference)
"""CASS block (LayerNorm + gradient-selected scan + fc1/dwconv/gelu/fc2 + residual)
on 8 TRN2 NeuronCores, pure data parallel over the batch.

Tensor-centric formulation: the depthwise 3-tap conv is folded into the fc1
matmul.  With rhs columns pre-scaled by the per-pixel LN rstd and two
augmented contraction rows (mu*rstd against -colsum(gamma*W1), and a ones row
against b1aug = beta@W1 + fc1_b, both zero at the conv pad columns), the fc1
PSUM accumulates, over 5 matmuls per block,

    psum[d, l] = sum_tau k_tau[d] * u[l+tau-1, d],   u = LN(x) @ W1 + b1,

i.e. the conv output directly.  The Scalar engine evacuates PSUM straight
through Gelu (bias = dw_b).  fc2 uses the gelu output as the stationary
operand so results come out pixel-major; the residual (+ x + fc2_b, preadded
host-side) is injected via an identity matmul into the same PSUM group.

Scheduling/efficiency notes (vs the first working version, 207us -> 177us):
 - warm-up matmuls at t=0 keep the PE HAM clock-gate at 2.4 GHz before the
   first real matmul and fill part of the prep window.
 - the aug-half taps are 3 direct K=66 matmuls against xt (6 matmuls per
   fc1 block).  The 5-matmul packed-xB variant is cheaper on paper but its
   2.1MB/sample of partition-shifted SBUF copies sit on the prep critical
   path and lose more to DMA-queue serialization than the 6th matmul costs
   (USE_XB toggles the packed variant back on).
 - fc2's residual is added by the DVE evacuation tensor_tensor (PSUM + xb)
   rather than identity matmuls: saves ~5us/sample of PE.
 - prep is split into per-half stages (stats -> transpose -> row-gather ->
   broadcast -> prescale) interleaved INTO the previous sample's fc1/fc2
   emission; fc1 emits block-major so pixel blocks 0..3 only gate on the
   h0 half; broadcast/prescale run in 3-4-tile column chunks so fc1 blocks
   unblock incrementally.
 - bn_stats (count, mean, count*var of even/odd element halves) is combined
   by ~13 strided vector ops per half instead of one bn_aggr per tile (each
   aggr pays a DVE pipeline-drain on its RAW dependency); the two halves'
   op chains are interleaved so the drain of one hides under the other.
   The mean lands doubled; the host halves the mu-aug weight row.  rstd is
   one division-free Newton step from z0 = 1.5 - 0.5v.
 - xb and y use partition-major HBM layouts (host transposes): the
   pixel-major rearrange moved 384-byte DMA packets which starve under
   packet-granularity round-robin against the large xt transfers.
 - xt half-1 ships only rows 0..65 (rows 66..127 are never read).
 - queue assignment: xb/y/aug/rstd-row on sync, xt on gpsimd (plus the
   partition_broadcasts, IRAM-warmed by a dummy broadcast at t=0), params +
   gelu evacuations on scalar, fc2 evacuation + stats on vector.
 - y is stored bf16 (host upcasts); halves the output DMA traffic.

The gradient selector: for uniform gamma the "gray" image mean_c(LN(x)) is a
constant, so grad_h = grad_v = 0, the MLP logits tie, softmax gives exactly
0.25 each in fp32, and argmax -> idx 0 for every sample: the 'v' (transpose)
branch is dead.  The device kernel therefore always scans row-major; a host
fallback handles non-uniform gamma by pre-transposing flagged samples."""

import numpy as np
import ml_dtypes

import concourse.mybir as mybir
import concourse.tile as tile
from concourse import bacc

B, H, W, C = 32, 56, 56, 192
D = 384                      # D_INNER
NCORES = 8
S = B // NCORES              # samples per core
L = H * W                    # 3136 pixels per sample
PT = 128                     # pixels per partition tile
NT = (L + PT - 1) // PT      # 25 pixel tiles (24 full + 64 tail)
TAIL = L - (NT - 1) * PT     # 64
NB = 448                     # fc1 N-block (one PSUM bank holds 448 f32)
NBLK = L // NB               # 7
LP = NT * PT                 # 3200: row-padded pixel count (25 full tiles)
EPS = 1e-5
USE_XB = False               # 5-matmul fc1 with pre-shifted xB pack
F32 = mybir.dt.float32
BF16 = mybir.dt.bfloat16
AL = mybir.AluOpType
AF = mybir.ActivationFunctionType

_CACHE = {}


def _build_nc(separate_stats: bool):
    nc = bacc.Bacc()
    # partition-major layouts: per-partition lines are contiguous so DMA
    # moves big packets (the pixel-major rearrange form moved 384-byte
    # packets and starved under packet-granularity queue round-robin)
    xb_d = nc.declare_dram_parameter("xb", [128, S, NT, C], BF16,
                                     isOutput=False)
    if separate_stats:
        xst_d = nc.declare_dram_parameter("xstat", [128, S, NT, C], BF16,
                                          isOutput=False)
    else:
        xst_d = xb_d
    xt0_d = nc.declare_dram_parameter("xt0", [S, 128, LP + 2], BF16,
                                      isOutput=False)
    xt1_d = nc.declare_dram_parameter("xt1", [S, 66, LP + 2], BF16,
                                      isOutput=False)
    w1a_d = nc.declare_dram_parameter("w1a", [128, 3, D], BF16, isOutput=False)
    if USE_XB:
        w1b1_d = nc.declare_dram_parameter("w1b1", [128, D], BF16,
                                           isOutput=False)
        w1b2_d = nc.declare_dram_parameter("w1b2", [70, D], BF16,
                                           isOutput=False)
    else:
        w1b_d = nc.declare_dram_parameter("w1b", [66, 3, D], BF16,
                                          isOutput=False)
    w2_d = nc.declare_dram_parameter("w2", [128, 3, C], BF16, isOutput=False)
    gb_d = nc.declare_dram_parameter("gb", [128, 3], F32, isOutput=False)
    id_d = nc.declare_dram_parameter("ident", [128, 128], F32, isOutput=False)
    y_d = nc.declare_dram_parameter("y", [128, S, NT, C], BF16, isOutput=True)

    with tile.TileContext(nc) as tc, \
         tc.tile_pool(name="const", bufs=1) as const, \
         tc.tile_pool(name="xb", bufs=4) as xbpool, \
         tc.tile_pool(name="xt", bufs=2) as xtpool, \
         tc.tile_pool(name="stat", bufs=4) as stat, \
         tc.tile_pool(name="rb", bufs=2) as rbpool, \
         tc.tile_pool(name="rr", bufs=2) as rrpool, \
         tc.tile_pool(name="xB", bufs=2) as xBpool, \
         tc.tile_pool(name="t", bufs=2) as tpool, \
         tc.tile_pool(name="y", bufs=3) as ypool, \
         tc.tile_pool(name="pf1", bufs=3, space="PSUM") as pf1, \
         tc.tile_pool(name="pf2", bufs=3, space="PSUM") as pf2, \
         tc.tile_pool(name="ptr", bufs=2, space="PSUM") as ptr:

        w1a = const.tile([128, 3, D], BF16)
        if USE_XB:
            w1b1 = const.tile([128, D], BF16)
            w1b2 = const.tile([70, D], BF16)
        else:
            w1b = const.tile([66, 3, D], BF16)
        w2 = const.tile([128, 3, C], BF16)
        gb = const.tile([128, 3], F32)
        ident = const.tile([128, 128], F32)
        wsrc = const.tile([128, 512], BF16)

        # wsrc memset first: warm-up matmuls depend only on it (no DMA)
        nc.vector.memset(wsrc, 0.0)
        nc.sync.dma_start(out=ident, in_=id_d[:, :])
        # warm the partition_broadcast ucode IRAM (~6us hidden first-use
        # cost) before the real broadcasts hit the critical path; input is
        # ident (already on chip) so it fires early
        bwarm = const.tile([128, 16], F32)
        nc.gpsimd.partition_broadcast(bwarm[:, :], ident[0:1, 0:16])

        def warm_mms(n, cols):
            # dummy matmuls: keep the PE HAM activity window busy so the
            # clock gate opens (and stays open) before real matmuls arrive
            wp = pf1.tile([128, NB], F32, tag="pt_")
            for _ in range(n):
                nc.tensor.matmul(wp[:, 0:cols], lhsT=wsrc[:, 0:128],
                                 rhs=wsrc[:, 0:cols], start=True, stop=True)

        state = {}

        def load(s):
            # pixel-major x (bf16): residual input + LN stats source
            xb_sb = xbpool.tile([128, NT, C], BF16)
            # split so the first h0 stats tiles unblock ~2us earlier
            nc.sync.dma_start(out=xb_sb[:, 0:8, :], in_=xb_d[:, s, 0:8, :])
            nc.sync.dma_start(out=xb_sb[:, 8:NT, :], in_=xb_d[:, s, 8:NT, :])
            if separate_stats:
                xs_sb = xbpool.tile([128, NT, C], BF16, tag="xstat")
                nc.scalar.dma_start(out=xs_sb, in_=xst_d[:, s, :, :])
            else:
                xs_sb = xb_sb
            # channel-major bf16 x with pad cols + aug-row slots; half-1
            # rows 66..127 are never read, so only 66 rows ship from HBM.
            # First two samples ride the scalar queue (gpsimd is busy with
            # the broadcast-ucode warmup during the fill).
            xt = xtpool.tile([128, 2, LP + 2], BF16)
            nc.gpsimd.dma_start(out=xt[:, 0, :], in_=xt0_d[s, :, :])
            nc.gpsimd.dma_start(out=xt[0:66, 1, :], in_=xt1_d[s, :, :])
            st = {"xb": xb_sb, "xs": xs_sb, "xt": xt}
            state[s] = st

        def _combine_ops(s, h):
            # Closures for the per-half stats combine + Newton rsqrt.  Each
            # half's 13 ops form a serial RAW chain; the caller interleaves
            # the independent h0/h1 chains so the DVE pipeline-drain wait of
            # one hides under the other's execution.
            # bn_stats emits (count, mean, count*var) for even/odd element
            # halves; mean lands DOUBLED in the pack mu row (the host halves
            # the matching aug weight row); var = (cv_e+cv_o)/C +
            # (m_e-m_o)^2/4.  rstd via one division-free Newton step from
            # z0 = 1.5 - 0.5 v (per-pixel var concentrates near 1).
            # pack[:,h,0,k] = 2*mu_k -> 2*mu*rstd, pack[:,h,1,k] = rstd
            st = state[s]
            bns, pack, scr = st["bns"], st["pack"], st["scr"]
            tlo, thi = (0, 16) if h == 0 else (16, NT)
            nt = thi - tlo
            d_ = scr[:, h, 0, 0:nt]
            s_ = scr[:, h, 1, 0:nt]
            dd = scr[:, h, 2, 0:nt]
            me = bns[:, tlo:thi, 1:2].rearrange("p t o -> p (t o)")
            mo = bns[:, tlo:thi, 4:5].rearrange("p t o -> p (t o)")
            cve = bns[:, tlo:thi, 2:3].rearrange("p t o -> p (t o)")
            cvo = bns[:, tlo:thi, 5:6].rearrange("p t o -> p (t o)")
            tt, ts = nc.vector.tensor_tensor, nc.vector.tensor_scalar
            mu = pack[:, h, 0, 0:nt]
            rs = pack[:, h, 1, 0:nt]
            return [
                lambda: tt(out=mu, in0=me, in1=mo, op=AL.add),
                lambda: tt(out=d_, in0=me, in1=mo, op=AL.subtract),
                lambda: tt(out=s_, in0=cve, in1=cvo, op=AL.add),
                lambda: tt(out=dd, in0=d_, in1=d_, op=AL.mult),
                lambda: ts(out=s_, in0=s_, scalar1=1.0 / C, scalar2=EPS,
                           op0=AL.mult, op1=AL.add),
                lambda: ts(out=dd, in0=dd, scalar1=0.25, scalar2=None,
                           op0=AL.mult),
                lambda: tt(out=s_, in0=s_, in1=dd, op=AL.add),
                lambda: ts(out=d_, in0=s_, scalar1=-0.5, scalar2=1.5,
                           op0=AL.mult, op1=AL.add),
                lambda: tt(out=dd, in0=d_, in1=d_, op=AL.mult),
                lambda: tt(out=dd, in0=dd, in1=s_, op=AL.mult),
                lambda: ts(out=dd, in0=dd, scalar1=-0.5, scalar2=1.5,
                           op0=AL.mult, op1=AL.add),
                lambda: tt(out=rs, in0=d_, in1=dd, op=AL.mult),
                lambda: tt(out=mu, in0=mu, in1=rs, op=AL.mult),
            ]

        def stats(s, h):
            st = state[s]
            xs_sb = st["xs"]
            if h == 0:
                bns = stat.tile([128, NT, 6], F32)
                pack = stat.tile([128, 2, 2, 16], F32)
                scr = stat.tile([128, 2, 3, 16], F32, tag="scr")
                nc.vector.memset(pack, 0.0)
                st["bns"], st["pack"], st["scr"] = bns, pack, scr
                for k in range(0, 16):
                    nc.vector.bn_stats(out=bns[:, k:k + 1, :],
                                       in_=xs_sb[:, k:k + 1, :])
            else:
                bns = st["bns"]
                pre = [lambda: nc.vector.memset(bns[TAIL:128, NT - 1:NT, :],
                                                0.0)]
                for k in range(16, NT - 1):
                    pre.append(lambda k=k: nc.vector.bn_stats(
                        out=bns[:, k:k + 1, :], in_=xs_sb[:, k:k + 1, :]))
                pre.append(lambda: nc.vector.bn_stats(
                    out=bns[0:TAIL, NT - 1:NT, :],
                    in_=xs_sb[0:TAIL, NT - 1:NT, :]))
                if s == 0:
                    # fill path: h0 combine zips with the h1 bn_stats so the
                    # first transpose isn't gated on the h1 stats
                    from itertools import zip_longest
                    for op0, op1 in zip_longest(pre, _combine_ops(s, 0)):
                        if op0 is not None:
                            op0()
                        if op1 is not None:
                            op1()
                    for op1 in _combine_ops(s, 1):
                        op1()
                else:
                    for op0 in pre:
                        op0()
                    for op0, op1 in zip(_combine_ops(s, 0),
                                        _combine_ops(s, 1)):
                        op0()
                        op1()

        def chain_a(s, h):
            # PE-transpose stats to rows, extract rstd row + mu*rstd aug row,
            # broadcast rstd to all partitions
            st = state[s]
            pack, xt = st["pack"], st["xt"]
            nt = 16 if h == 0 else NT - 16
            clo, chi = (0, 16 * PT) if h == 0 else (16 * PT, NT * PT)
            if h == 0:
                rrow = rrpool.tile([1, LP], BF16)
                rstd_b = rbpool.tile([128, LP], BF16)
                st["rrow"], st["rstd_b"] = rrow, rstd_b
                if USE_XB:
                    st["xB"] = xBpool.tile([128, 2, LP + 2], BF16, name="xB")
            else:
                rrow, rstd_b = st["rrow"], st["rstd_b"]
            tpp = ptr.tile([32, 128], F32)
            nc.tensor.transpose(
                out=tpp[0:32, :],
                in_=pack[:, h, :, :].rearrange("p a b -> p (a b)"),
                identity=ident)
            packT = stat.tile([32, 128], BF16, tag=f"pT{h}")
            nc.vector.tensor_copy(out=packT, in_=tpp)
            # mu*rstd aug row (row 64 of half 1); ones row host-prepared
            nc.sync.dma_start(out=xt[64:65, 1, 1 + clo:1 + chi],
                              in_=packT[0:nt, :])
            # rstd row-gather + broadcast in column chunks so each
            # broadcast gates only on its own small DMA and downstream
            # prescale/fc1 blocks unblock incrementally
            tw = 4 if h == 0 else 3
            for j in range(0, nt, tw):
                jw = min(tw, nt - j)
                qlo, cw = clo + j * PT, jw * PT
                nc.sync.dma_start(out=rrow[0:1, qlo:qlo + cw],
                                  in_=packT[16 + j:16 + j + jw, :])
                nc.gpsimd.partition_broadcast(rstd_b[:, qlo:qlo + cw],
                                              rrow[0:1, qlo:qlo + cw])

        def chain_b(s, h):
            # prescale xt by rstd in place (aug rows 64/65 excluded), in
            # column chunks matching the broadcast chunks
            st = state[s]
            xt, rstd_b = st["xt"], st["rstd_b"]
            clo, chi = (0, 16 * PT) if h == 0 else (16 * PT, NT * PT)
            nt = (chi - clo) // PT
            tw = 4 if h == 0 else 3
            for j in range(0, nt, tw):
                jw = min(tw, nt - j)
                qlo, cw = clo + j * PT, jw * PT
                nc.vector.tensor_tensor(out=xt[:, 0, 1 + qlo:1 + qlo + cw],
                                        in0=xt[:, 0, 1 + qlo:1 + qlo + cw],
                                        in1=rstd_b[:, qlo:qlo + cw],
                                        op=AL.mult)
                nc.vector.tensor_tensor(out=xt[0:64, 1, 1 + qlo:1 + qlo + cw],
                                        in0=xt[0:64, 1, 1 + qlo:1 + qlo + cw],
                                        in1=rstd_b[0:64, qlo:qlo + cw],
                                        op=AL.mult)
            if USE_XB:
                xB = st["xB"]
                if h == 0:
                    nc.sync.dma_start(out=xB[0:66, 0, 1:2 + chi],
                                      in_=xt[0:66, 1, 0:1 + chi])
                    nc.sync.dma_start(out=xB[66:128, 0, 0:1 + chi],
                                      in_=xt[0:62, 1, 0:1 + chi])
                    nc.sync.dma_start(out=xB[0:4, 1, 0:1 + chi],
                                      in_=xt[62:66, 1, 0:1 + chi])
                    nc.sync.dma_start(out=xB[4:70, 1, 0:chi],
                                      in_=xt[0:66, 1, 1:1 + chi])
                else:
                    nc.sync.dma_start(out=xB[0:66, 0, 2 + clo:LP + 2],
                                      in_=xt[0:66, 1, 1 + clo:LP + 1])
                    nc.sync.dma_start(out=xB[66:128, 0, 1 + clo:LP + 2],
                                      in_=xt[0:62, 1, 1 + clo:LP + 2])
                    nc.sync.dma_start(out=xB[0:4, 1, 1 + clo:LP + 2],
                                      in_=xt[62:66, 1, 1 + clo:LP + 2])
                    nc.sync.dma_start(out=xB[4:70, 1, clo:LP + 1],
                                      in_=xt[0:66, 1, 1 + clo:LP + 2])

        def fc1(s, blks):
            # fc1 + conv fused: 6 accumulating matmuls per psum block (3 taps
            # x 2 channel chunks, the aug-half read directly from xt with the
            # tap's column shift -- no shifted-copy build), then Gelu(psum +
            # dw_b) evacuates PSUM directly.  Block-major so the h0 pixel
            # blocks only depend on the h0 half of the prep chain.
            st = state[s]
            xt = st["xt"]
            if "t" not in st:
                st["t"] = tpool.tile([128, 3, L], BF16, name="t")
            t = st["t"]
            xB = st.get("xB")
            for blk in blks:
                cs = blk * NB
                for m in range(3):
                    pt_ = pf1.tile([128, NB], F32)
                    for tau in range(3):
                        nc.tensor.matmul(
                            pt_, lhsT=w1a[:, tau, m * 128:(m + 1) * 128],
                            rhs=xt[:, 0, cs + tau: cs + tau + NB],
                            start=(tau == 0), stop=False)
                    if USE_XB:
                        nc.tensor.matmul(
                            pt_, lhsT=w1b1[:, m * 128:(m + 1) * 128],
                            rhs=xB[:, 0, cs + 1: cs + 1 + NB],
                            start=False, stop=False)
                        nc.tensor.matmul(
                            pt_, lhsT=w1b2[0:70, m * 128:(m + 1) * 128],
                            rhs=xB[0:70, 1, cs + 1: cs + 1 + NB],
                            start=False, stop=True)
                    else:
                        for tau in range(3):
                            nc.tensor.matmul(
                                pt_,
                                lhsT=w1b[0:66, tau, m * 128:(m + 1) * 128],
                                rhs=xt[0:66, 1, cs + tau: cs + tau + NB],
                                start=False, stop=(tau == 2))
                    nc.scalar.activation(out=t[:, m, cs:cs + NB], in_=pt_,
                                         func=AF.Gelu, bias=gb[:, m:m + 1],
                                         scale=1.0)

        def fc2_g(s, g):
            # fc2 (stationary = gelu output -> pixel-major out); the
            # residual (x + fc2_b, preadded host-side) is added by the DVE
            # evacuation tensor_tensor reading PSUM + xb directly -- no
            # identity matmuls on the PE
            st = state[s]
            xb_sb, t = st["xb"], st["t"]
            y_sb = ypool.tile([128, 6, C], BF16)
            for jp in range(3):
                kp = 3 * g + jp
                py = pf2.tile([128, 2, C], F32)
                for j in range(2):
                    k = 2 * kp + j
                    for kc in range(3):
                        nc.tensor.matmul(
                            py[:, j, :],
                            lhsT=t[:, kc, k * PT:(k + 1) * PT],
                            rhs=w2[:, kc, :],
                            start=(kc == 0), stop=(kc == 2))
                nc.vector.tensor_tensor(out=y_sb[:, 2 * jp:2 * jp + 2, :],
                                        in0=py,
                                        in1=xb_sb[:, 6 * g + 2 * jp:
                                                  6 * g + 2 * jp + 2, :],
                                        op=AL.add)
            nc.sync.dma_start(out=y_d[:, s, 6 * g:6 * g + 6, :], in_=y_sb)

        def fc2_tail(s):
            # tail pixel tile (64 rows)
            st = state.pop(s)
            xb_sb, t = st["xb"], st["t"]
            py = pf2.tile([128, 2, C], F32)
            for kc in range(3):
                nc.tensor.matmul(py[0:TAIL, 0, :],
                                 lhsT=t[:, kc, (NT - 1) * PT: L],
                                 rhs=w2[:, kc, :],
                                 start=(kc == 0), stop=(kc == 2))
            y_sb = ypool.tile([128, 6, C], BF16, tag="ytail")
            nc.vector.tensor_tensor(out=y_sb[0:TAIL, 0, :],
                                    in0=py[0:TAIL, 0, :],
                                    in1=xb_sb[0:TAIL, NT - 1, :], op=AL.add)
            nc.sync.dma_start(out=y_d[0:TAIL, s, NT - 1, :],
                              in_=y_sb[0:TAIL, 0, :])

        # ---- emission schedule: prep stages interleave into the previous
        # sample's fc1/fc2 so every engine queue sees ops in dependency
        # order.  x loads go out before params (params aren't needed until
        # the first matmul ~25us in).
        warm_mms(36, 448)
        load(0)
        nc.scalar.dma_start(out=w1a, in_=w1a_d[:, :, :])
        if USE_XB:
            nc.scalar.dma_start(out=w1b1, in_=w1b1_d[:, :])
            nc.scalar.dma_start(out=w1b2, in_=w1b2_d[:, :])
        else:
            nc.scalar.dma_start(out=w1b, in_=w1b_d[:, :, :])
        nc.sync.dma_start(out=w2, in_=w2_d[:, :, :])
        nc.sync.dma_start(out=gb, in_=gb_d[:, :])
        # preload the gelu activation table
        warm = const.tile([128, 1], BF16)
        nc.scalar.activation(out=warm, in_=gb[:, 0:1], func=AF.Gelu,
                             bias=0.0, scale=1.0)
        stats(0, 0)
        stats(0, 1)
        warm_mms(16, 448)
        chain_a(0, 0)
        chain_a(0, 1)
        warm_mms(44, 448)
        chain_b(0, 0)
        chain_b(0, 1)
        if S > 1:
            load(1)
            stats(1, 0)
        for s in range(S):
            nxt = s + 1
            if nxt < S:
                fc1(s, range(0, 4))
                stats(nxt, 1)
                chain_a(nxt, 0)
                fc1(s, range(4, 6))
                chain_a(nxt, 1)
                fc1(s, range(6, NBLK))
                fc2_g(s, 0)
                chain_b(nxt, 0)
                fc2_g(s, 1)
                chain_b(nxt, 1)
                fc2_g(s, 2)
                fc2_g(s, 3)
                fc2_tail(s)
            else:
                fc1(s, range(0, NBLK))
                for g in range(4):
                    fc2_g(s, g)
                fc2_tail(s)
            if nxt + 1 < S:
                load(nxt + 1)
                stats(nxt + 1, 0)
    nc.finalize()
    return nc


def _get_nc(separate_stats=False):
    key = ("nc", separate_stats)
    if key not in _CACHE:
        _CACHE[key] = _build_nc(separate_stats)
    return _CACHE[key]


def _host_params(gamma, beta, fc1_w, fc1_b, dw_w, dw_b, fc2_w, fc2_b):
    bf = ml_dtypes.bfloat16
    w1g = (fc1_w * gamma[:, None]).astype(np.float32)          # [192, 384]
    s1g = w1g.sum(0)                                           # [384]
    b1aug = (beta @ fc1_w + fc1_b).astype(np.float32)          # [384]
    # mu aug row arrives doubled from the device stats combine -> halve here
    wfull = np.concatenate([w1g, -0.5 * s1g[None, :], b1aug[None, :]],
                           0)  # [194, D]
    k = dw_w[:, 0, :].astype(np.float32)                       # [384, 3]
    w1a = np.zeros((128, 3, D), dtype=bf)
    w1b = np.zeros((66, 3, D), dtype=bf)
    for tau in range(3):
        wt = wfull * k[:, tau][None, :]
        w1a[:, tau, :] = wt[0:128].astype(bf)
        w1b[:, tau, :] = wt[128:194].astype(bf)    # 66 aug-half rows per tap
    w2 = np.ascontiguousarray(
        fc2_w.reshape(3, 128, C).transpose(1, 0, 2)).astype(bf)  # [128,3,192]
    gb = np.ascontiguousarray(
        dw_b.reshape(3, 128).T).astype(np.float32)               # [128, 3]
    ident = np.eye(128, dtype=np.float32)
    params = dict(w1a=w1a, w2=w2, gb=gb, ident=ident)
    if USE_XB:
        params["w1b1"] = np.concatenate([w1b[:, 0, :], w1b[0:62, 1, :]], 0)
        params["w1b2"] = np.concatenate([w1b[62:66, 1, :], w1b[:, 2, :]], 0)
    else:
        params["w1b"] = w1b
    return params


def _host_xt(x_dev):
    """Channel-major bf16 copy of x with zero pad columns at 0 and L+1.
    xt0 [nb, 128, L+2] = channels 0..127; xt1 [nb, 66, L+2]: rows 0..63 =
    channels 128..191, row 64 = mu*rstd slot (runtime), row 65 = ones row
    (set here, zero at the pads)."""
    bf = ml_dtypes.bfloat16
    nb = x_dev.shape[0]
    arr = np.ascontiguousarray(
        x_dev.reshape(nb, L, C).transpose(0, 2, 1)).astype(bf)  # [nb, 192, L]
    xt0 = np.zeros((nb, 128, LP + 2), dtype=bf)
    xt1 = np.zeros((nb, 66, LP + 2), dtype=bf)
    xt0[:, :, 1:L + 1] = arr[:, 0:128]
    xt1[:, 0:64, 1:L + 1] = arr[:, 128:192]
    xt1[:, 65, 1:L + 1] = 1.0
    return xt0, xt1


def _selector_flags(x, gamma, beta, sel_w1, sel_b1, sel_w2, sel_b2):
    """Exact numpy replica of the reference direction selector. Only used
    when gamma is non-uniform (otherwise the scores tie and idx==0 always)."""
    xf = x.astype(np.float32)
    mu = xf.mean(-1, keepdims=True)
    var = ((xf - mu) ** 2).mean(-1, keepdims=True)
    xn = (xf - mu) / np.sqrt(var + EPS) * gamma + beta
    xg = xn.mean(-1)
    gh = np.abs(xg[:, :, 1:] - xg[:, :, :-1]).mean(axis=(1, 2))
    gv = np.abs(xg[:, 1:, :] - xg[:, :-1, :]).mean(axis=(1, 2))
    scores = np.stack([gh, gv, 0.8 * (gh + gv) * 0.5, np.abs(gh - gv)], 1)
    hdn = np.maximum(scores @ sel_w1 + sel_b1, 0.0)
    logits = hdn @ sel_w2 + sel_b2
    ex = np.exp(logits - logits.max(1, keepdims=True))
    probs = ex / ex.sum(1, keepdims=True)
    return probs.argmax(1) % 4 == 1


def build_in_maps(inputs):
    """Shared by kernel() and test harnesses: host preprocessing + sharding.
    Returns (in_maps, x, x_dev, flags)."""
    x = np.asarray(inputs["x"], dtype=np.float32)
    gamma = np.asarray(inputs["gamma"], np.float32)
    beta = np.asarray(inputs["beta"], np.float32)
    fc2_b = np.asarray(inputs["fc2_b"], np.float32)
    params = _host_params(
        gamma, beta,
        np.asarray(inputs["fc1_w"], np.float32),
        np.asarray(inputs["fc1_b"], np.float32),
        np.asarray(inputs["dw_w"], np.float32),
        np.asarray(inputs["dw_b"], np.float32),
        np.asarray(inputs["fc2_w"], np.float32),
        fc2_b,
    )

    # Routing: uniform gamma => gray image is constant => scores tie => idx 0
    # for every sample (see module docstring).  Otherwise compute the selector
    # on host and pre-transpose flagged samples (mathematically exact fixup).
    if np.ptp(gamma) == 0.0:
        flags = np.zeros(B, dtype=bool)
    else:
        flags = _selector_flags(
            x, gamma, beta,
            np.asarray(inputs["sel_w1"], np.float32),
            np.asarray(inputs["sel_b1"], np.float32),
            np.asarray(inputs["sel_w2"], np.float32),
            np.asarray(inputs["sel_b2"], np.float32))
    x_dev = x
    if flags.any():
        x_dev = x.copy()
        x_dev[flags] = np.swapaxes(x_dev[flags], 1, 2)

    separate_stats = bool(np.any(fc2_b != 0.0))
    xt0, xt1 = _host_xt(x_dev)
    xb = x_dev + fc2_b

    def _pmajor(v):
        # [S, H, W, C] -> [128, S, NT, C] partition-major pixel tiles
        bf = ml_dtypes.bfloat16
        v = v.reshape(-1, L, C)
        out = np.zeros((128, v.shape[0], NT, C), dtype=bf)
        out[:, :, 0:NT - 1, :] = (
            v[:, 0:(NT - 1) * PT].reshape(-1, NT - 1, PT, C)
            .transpose(2, 0, 1, 3))
        out[0:TAIL, :, NT - 1, :] = (
            v[:, (NT - 1) * PT:L].transpose(1, 0, 2))
        return out

    in_maps = []
    for i in range(NCORES):
        sl = slice(S * i, S * (i + 1))
        m = {"xb": _pmajor(xb[sl]),
             "xt0": xt0[sl],
             "xt1": xt1[sl]}
        if separate_stats:
            m["xstat"] = _pmajor(x_dev[sl])
        m.update(params)
        in_maps.append(m)
    return in_maps, x, x_dev, flags


def kernel(**inputs):
    from concourse.bass_utils import run_bass_kernel_spmd

    in_maps, x, x_dev, flags = build_in_maps(inputs)
    separate_stats = "xstat" in in_maps[0]
    nc = _get_nc(separate_stats)
    res = run_bass_kernel_spmd(nc, in_maps, list(range(NCORES)))
    # y arrives partition-major [128, S, NT, C]; un-transpose on host
    y = np.concatenate(
        [r["y"].astype(np.float32).transpose(1, 2, 0, 3)
             .reshape(S, NT * PT, C)[:, :L, :].reshape(S, H, W, C)
         for r in res.results], 0)
    if flags.any():
        # device computed x_dev + F(x_dev); reference wants x + F(x_dev)
        # (row-major unscan orientation is identical)
        y = x + (y - x_dev)
    return y.astype(np.float32)


# revision 75
# speedup vs baseline: 1.0099x; 1.0099x over previous
"""CASS block (LayerNorm + gradient-selected scan + fc1/dwconv/gelu/fc2 + residual)
on 8 TRN2 NeuronCores, pure data parallel over the batch.

Tensor-centric formulation: the depthwise 3-tap conv is folded into the fc1
matmul.  With rhs columns pre-scaled by the per-pixel LN rstd and two
augmented contraction rows (mu*rstd against -colsum(gamma*W1), and a ones row
against b1aug = beta@W1 + fc1_b, both zero at the conv pad columns), the fc1
PSUM accumulates, over 5 matmuls per block,

    psum[d, l] = sum_tau k_tau[d] * u[l+tau-1, d],   u = LN(x) @ W1 + b1,

i.e. the conv output directly.  The Scalar engine evacuates PSUM straight
through Gelu (bias = dw_b).  fc2 uses the gelu output as the stationary
operand so results come out pixel-major; the residual (+ x + fc2_b, preadded
host-side) is injected via an identity matmul into the same PSUM group.

Scheduling/efficiency notes (vs the first working version, 207us -> 177us):
 - warm-up matmuls at t=0 keep the PE HAM clock-gate at 2.4 GHz before the
   first real matmul and fill part of the prep window.
 - the aug-half taps are 3 direct K=66 matmuls against xt (6 matmuls per
   fc1 block).  The 5-matmul packed-xB variant is cheaper on paper but its
   2.1MB/sample of partition-shifted SBUF copies sit on the prep critical
   path and lose more to DMA-queue serialization than the 6th matmul costs
   (USE_XB toggles the packed variant back on).
 - fc2's residual is added by the DVE evacuation tensor_tensor (PSUM + xb)
   rather than identity matmuls: saves ~5us/sample of PE.
 - prep is split into per-half stages (stats -> transpose -> row-gather ->
   broadcast -> prescale) interleaved INTO the previous sample's fc1/fc2
   emission; fc1 emits block-major so pixel blocks 0..3 only gate on the
   h0 half; broadcast/prescale run in 3-4-tile column chunks so fc1 blocks
   unblock incrementally.
 - bn_stats (count, mean, count*var of even/odd element halves) is combined
   by ~13 strided vector ops per half instead of one bn_aggr per tile (each
   aggr pays a DVE pipeline-drain on its RAW dependency); the two halves'
   op chains are interleaved so the drain of one hides under the other.
   The mean lands doubled; the host halves the mu-aug weight row.  rstd is
   one division-free Newton step from z0 = 1.5 - 0.5v.
 - xb and y use partition-major HBM layouts (host transposes): the
   pixel-major rearrange moved 384-byte DMA packets which starve under
   packet-granularity round-robin against the large xt transfers.
 - xt half-1 ships only rows 0..65 (rows 66..127 are never read).
 - queue assignment: xb/y/aug/rstd-row on sync, xt on gpsimd (plus the
   partition_broadcasts, IRAM-warmed by a dummy broadcast at t=0), params +
   gelu evacuations on scalar, fc2 evacuation + stats on vector.
 - y is stored bf16 (host upcasts); halves the output DMA traffic.

The gradient selector: for uniform gamma the "gray" image mean_c(LN(x)) is a
constant, so grad_h = grad_v = 0, the MLP logits tie, softmax gives exactly
0.25 each in fp32, and argmax -> idx 0 for every sample: the 'v' (transpose)
branch is dead.  The device kernel therefore always scans row-major; a host
fallback handles non-uniform gamma by pre-transposing flagged samples."""

import numpy as np
import ml_dtypes

import concourse.mybir as mybir
import concourse.tile as tile
from concourse import bacc

B, H, W, C = 32, 56, 56, 192
D = 384                      # D_INNER
NCORES = 8
S = B // NCORES              # samples per core
L = H * W                    # 3136 pixels per sample
PT = 128                     # pixels per partition tile
NT = (L + PT - 1) // PT      # 25 pixel tiles (24 full + 64 tail)
TAIL = L - (NT - 1) * PT     # 64
NB = 448                     # fc1 N-block (one PSUM bank holds 448 f32)
NBLK = L // NB               # 7
LP = NT * PT                 # 3200: row-padded pixel count (25 full tiles)
EPS = 1e-5
USE_XB = False               # 5-matmul fc1 with pre-shifted xB pack
F32 = mybir.dt.float32
BF16 = mybir.dt.bfloat16
AL = mybir.AluOpType
AF = mybir.ActivationFunctionType

_CACHE = {}


def _build_nc(separate_stats: bool):
    nc = bacc.Bacc()
    # partition-major layouts: per-partition lines are contiguous so DMA
    # moves big packets (the pixel-major rearrange form moved 384-byte
    # packets and starved under packet-granularity queue round-robin)
    xb_d = nc.declare_dram_parameter("xb", [128, S, NT, C], BF16,
                                     isOutput=False)
    if separate_stats:
        xst_d = nc.declare_dram_parameter("xstat", [128, S, NT, C], BF16,
                                          isOutput=False)
    else:
        xst_d = xb_d
    xt0_d = nc.declare_dram_parameter("xt0", [S, 128, LP + 2], BF16,
                                      isOutput=False)
    xt1_d = nc.declare_dram_parameter("xt1", [S, 66, LP + 2], BF16,
                                      isOutput=False)
    w1a_d = nc.declare_dram_parameter("w1a", [128, 3, D], BF16, isOutput=False)
    if USE_XB:
        w1b1_d = nc.declare_dram_parameter("w1b1", [128, D], BF16,
                                           isOutput=False)
        w1b2_d = nc.declare_dram_parameter("w1b2", [70, D], BF16,
                                           isOutput=False)
    else:
        w1b_d = nc.declare_dram_parameter("w1b", [66, 3, D], BF16,
                                          isOutput=False)
    w2_d = nc.declare_dram_parameter("w2", [128, 3, C], BF16, isOutput=False)
    gb_d = nc.declare_dram_parameter("gb", [128, 3], F32, isOutput=False)
    id_d = nc.declare_dram_parameter("ident", [128, 128], F32, isOutput=False)
    y_d = nc.declare_dram_parameter("y", [128, S, NT, C], BF16, isOutput=True)

    with tile.TileContext(nc) as tc, \
         tc.tile_pool(name="const", bufs=1) as const, \
         tc.tile_pool(name="xb", bufs=4) as xbpool, \
         tc.tile_pool(name="xt", bufs=2) as xtpool, \
         tc.tile_pool(name="stat", bufs=4) as stat, \
         tc.tile_pool(name="rb", bufs=2) as rbpool, \
         tc.tile_pool(name="rr", bufs=2) as rrpool, \
         tc.tile_pool(name="xB", bufs=2) as xBpool, \
         tc.tile_pool(name="t", bufs=2) as tpool, \
         tc.tile_pool(name="y", bufs=3) as ypool, \
         tc.tile_pool(name="pf1", bufs=3, space="PSUM") as pf1, \
         tc.tile_pool(name="pf2", bufs=3, space="PSUM") as pf2, \
         tc.tile_pool(name="ptr", bufs=2, space="PSUM") as ptr:

        w1a = const.tile([128, 3, D], BF16)
        if USE_XB:
            w1b1 = const.tile([128, D], BF16)
            w1b2 = const.tile([70, D], BF16)
        else:
            w1b = const.tile([66, 3, D], BF16)
        w2 = const.tile([128, 3, C], BF16)
        gb = const.tile([128, 3], F32)
        ident = const.tile([128, 128], F32)
        wsrc = const.tile([128, 512], BF16)

        # wsrc memset first: warm-up matmuls depend only on it (no DMA)
        nc.vector.memset(wsrc, 0.0)
        nc.sync.dma_start(out=ident, in_=id_d[:, :])
        # warm the partition_broadcast ucode IRAM (~6us hidden first-use
        # cost) before the real broadcasts hit the critical path; input is
        # ident (already on chip) so it fires early
        bwarm = const.tile([128, 16], F32)
        nc.gpsimd.partition_broadcast(bwarm[:, :], ident[0:1, 0:16])

        def warm_mms(n, cols):
            # dummy matmuls: keep the PE HAM activity window busy so the
            # clock gate opens (and stays open) before real matmuls arrive
            wp = pf1.tile([128, NB], F32, tag="pt_")
            for _ in range(n):
                nc.tensor.matmul(wp[:, 0:cols], lhsT=wsrc[:, 0:128],
                                 rhs=wsrc[:, 0:cols], start=True, stop=True)

        state = {}

        def load(s):
            # pixel-major x (bf16): residual input + LN stats source
            xb_sb = xbpool.tile([128, NT, C], BF16)
            # split at the half boundary: the h0 stats (which gate the whole
            # prep chain) unblock without waiting for the h1 tiles' bytes
            nc.sync.dma_start(out=xb_sb[:, 0:16, :], in_=xb_d[:, s, 0:16, :])
            nc.sync.dma_start(out=xb_sb[:, 16:NT, :],
                              in_=xb_d[:, s, 16:NT, :])
            if separate_stats:
                xs_sb = xbpool.tile([128, NT, C], BF16, tag="xstat")
                nc.scalar.dma_start(out=xs_sb, in_=xst_d[:, s, :, :])
            else:
                xs_sb = xb_sb
            # channel-major bf16 x with pad cols + aug-row slots; half-1
            # rows 66..127 are never read, so only 66 rows ship from HBM.
            # First two samples ride the scalar queue (gpsimd is busy with
            # the broadcast-ucode warmup during the fill).
            xt = xtpool.tile([128, 2, LP + 2], BF16)
            nc.gpsimd.dma_start(out=xt[:, 0, :], in_=xt0_d[s, :, :])
            nc.gpsimd.dma_start(out=xt[0:66, 1, :], in_=xt1_d[s, :, :])
            st = {"xb": xb_sb, "xs": xs_sb, "xt": xt}
            state[s] = st

        def _combine_ops(s, h):
            # Closures for the per-half stats combine + Newton rsqrt.  Each
            # half's 13 ops form a serial RAW chain; the caller interleaves
            # the independent h0/h1 chains so the DVE pipeline-drain wait of
            # one hides under the other's execution.
            # bn_stats emits (count, mean, count*var) for even/odd element
            # halves; mean lands DOUBLED in the pack mu row (the host halves
            # the matching aug weight row); var = (cv_e+cv_o)/C +
            # (m_e-m_o)^2/4.  rstd via one division-free Newton step from
            # z0 = 1.5 - 0.5 v (per-pixel var concentrates near 1).
            # pack[:,h,0,k] = 2*mu_k -> 2*mu*rstd, pack[:,h,1,k] = rstd
            st = state[s]
            bns, pack, scr = st["bns"], st["pack"], st["scr"]
            tlo, thi = (0, 16) if h == 0 else (16, NT)
            nt = thi - tlo
            d_ = scr[:, h, 0, 0:nt]
            s_ = scr[:, h, 1, 0:nt]
            dd = scr[:, h, 2, 0:nt]
            me = bns[:, tlo:thi, 1:2].rearrange("p t o -> p (t o)")
            mo = bns[:, tlo:thi, 4:5].rearrange("p t o -> p (t o)")
            cve = bns[:, tlo:thi, 2:3].rearrange("p t o -> p (t o)")
            cvo = bns[:, tlo:thi, 5:6].rearrange("p t o -> p (t o)")
            tt, ts = nc.vector.tensor_tensor, nc.vector.tensor_scalar
            mu = pack[:, h, 0, 0:nt]
            rs = pack[:, h, 1, 0:nt]
            return [
                lambda: tt(out=mu, in0=me, in1=mo, op=AL.add),
                lambda: tt(out=d_, in0=me, in1=mo, op=AL.subtract),
                lambda: tt(out=s_, in0=cve, in1=cvo, op=AL.add),
                lambda: tt(out=dd, in0=d_, in1=d_, op=AL.mult),
                lambda: ts(out=s_, in0=s_, scalar1=1.0 / C, scalar2=EPS,
                           op0=AL.mult, op1=AL.add),
                lambda: ts(out=dd, in0=dd, scalar1=0.25, scalar2=None,
                           op0=AL.mult),
                lambda: tt(out=s_, in0=s_, in1=dd, op=AL.add),
                lambda: ts(out=d_, in0=s_, scalar1=-0.5, scalar2=1.5,
                           op0=AL.mult, op1=AL.add),
                lambda: tt(out=dd, in0=d_, in1=d_, op=AL.mult),
                lambda: tt(out=dd, in0=dd, in1=s_, op=AL.mult),
                lambda: ts(out=dd, in0=dd, scalar1=-0.5, scalar2=1.5,
                           op0=AL.mult, op1=AL.add),
                lambda: tt(out=rs, in0=d_, in1=dd, op=AL.mult),
                lambda: tt(out=mu, in0=mu, in1=rs, op=AL.mult),
            ]

        def stats(s, h):
            st = state[s]
            xs_sb = st["xs"]
            if h == 0:
                bns = stat.tile([128, NT, 6], F32)
                pack = stat.tile([128, 2, 2, 16], F32)
                scr = stat.tile([128, 2, 3, 16], F32, tag="scr")
                nc.vector.memset(pack, 0.0)
                st["bns"], st["pack"], st["scr"] = bns, pack, scr
                for k in range(0, 16):
                    nc.vector.bn_stats(out=bns[:, k:k + 1, :],
                                       in_=xs_sb[:, k:k + 1, :])
            else:
                bns = st["bns"]
                pre = [lambda: nc.vector.memset(bns[TAIL:128, NT - 1:NT, :],
                                                0.0)]
                for k in range(16, NT - 1):
                    pre.append(lambda k=k: nc.vector.bn_stats(
                        out=bns[:, k:k + 1, :], in_=xs_sb[:, k:k + 1, :]))
                pre.append(lambda: nc.vector.bn_stats(
                    out=bns[0:TAIL, NT - 1:NT, :],
                    in_=xs_sb[0:TAIL, NT - 1:NT, :]))
                if s == 0:
                    # fill path: h0 combine zips with the h1 bn_stats so the
                    # first transpose isn't gated on the h1 stats
                    from itertools import zip_longest
                    for op0, op1 in zip_longest(pre, _combine_ops(s, 0)):
                        if op0 is not None:
                            op0()
                        if op1 is not None:
                            op1()
                    for op1 in _combine_ops(s, 1):
                        op1()
                else:
                    for op0 in pre:
                        op0()
                    for op0, op1 in zip(_combine_ops(s, 0),
                                        _combine_ops(s, 1)):
                        op0()
                        op1()

        def chain_a(s, h):
            # PE-transpose stats to rows, extract rstd row + mu*rstd aug row,
            # broadcast rstd to all partitions
            st = state[s]
            pack, xt = st["pack"], st["xt"]
            nt = 16 if h == 0 else NT - 16
            clo, chi = (0, 16 * PT) if h == 0 else (16 * PT, NT * PT)
            if h == 0:
                rrow = rrpool.tile([1, LP], BF16)
                rstd_b = rbpool.tile([128, LP], BF16)
                st["rrow"], st["rstd_b"] = rrow, rstd_b
                if USE_XB:
                    st["xB"] = xBpool.tile([128, 2, LP + 2], BF16, name="xB")
            else:
                rrow, rstd_b = st["rrow"], st["rstd_b"]
            tpp = ptr.tile([32, 128], F32)
            nc.tensor.transpose(
                out=tpp[0:32, :],
                in_=pack[:, h, :, :].rearrange("p a b -> p (a b)"),
                identity=ident)
            packT = stat.tile([32, 128], BF16, tag=f"pT{h}")
            nc.vector.tensor_copy(out=packT, in_=tpp)
            # mu*rstd aug row (row 64 of half 1); ones row host-prepared
            nc.sync.dma_start(out=xt[64:65, 1, 1 + clo:1 + chi],
                              in_=packT[0:nt, :])
            # rstd row-gather + broadcast in column chunks so each
            # broadcast gates only on its own small DMA and downstream
            # prescale/fc1 blocks unblock incrementally
            tw = 4 if h == 0 else 3
            for j in range(0, nt, tw):
                jw = min(tw, nt - j)
                qlo, cw = clo + j * PT, jw * PT
                nc.sync.dma_start(out=rrow[0:1, qlo:qlo + cw],
                                  in_=packT[16 + j:16 + j + jw, :])
                nc.gpsimd.partition_broadcast(rstd_b[:, qlo:qlo + cw],
                                              rrow[0:1, qlo:qlo + cw])

        def chain_b(s, h):
            # prescale xt by rstd in place (aug rows 64/65 excluded), in
            # column chunks matching the broadcast chunks
            st = state[s]
            xt, rstd_b = st["xt"], st["rstd_b"]
            clo, chi = (0, 16 * PT) if h == 0 else (16 * PT, NT * PT)
            nt = (chi - clo) // PT
            tw = 4 if h == 0 else 3
            for j in range(0, nt, tw):
                jw = min(tw, nt - j)
                qlo, cw = clo + j * PT, jw * PT
                nc.vector.tensor_tensor(out=xt[:, 0, 1 + qlo:1 + qlo + cw],
                                        in0=xt[:, 0, 1 + qlo:1 + qlo + cw],
                                        in1=rstd_b[:, qlo:qlo + cw],
                                        op=AL.mult)
                nc.vector.tensor_tensor(out=xt[0:64, 1, 1 + qlo:1 + qlo + cw],
                                        in0=xt[0:64, 1, 1 + qlo:1 + qlo + cw],
                                        in1=rstd_b[0:64, qlo:qlo + cw],
                                        op=AL.mult)
            if USE_XB:
                xB = st["xB"]
                if h == 0:
                    nc.sync.dma_start(out=xB[0:66, 0, 1:2 + chi],
                                      in_=xt[0:66, 1, 0:1 + chi])
                    nc.sync.dma_start(out=xB[66:128, 0, 0:1 + chi],
                                      in_=xt[0:62, 1, 0:1 + chi])
                    nc.sync.dma_start(out=xB[0:4, 1, 0:1 + chi],
                                      in_=xt[62:66, 1, 0:1 + chi])
                    nc.sync.dma_start(out=xB[4:70, 1, 0:chi],
                                      in_=xt[0:66, 1, 1:1 + chi])
                else:
                    nc.sync.dma_start(out=xB[0:66, 0, 2 + clo:LP + 2],
                                      in_=xt[0:66, 1, 1 + clo:LP + 1])
                    nc.sync.dma_start(out=xB[66:128, 0, 1 + clo:LP + 2],
                                      in_=xt[0:62, 1, 1 + clo:LP + 2])
                    nc.sync.dma_start(out=xB[0:4, 1, 1 + clo:LP + 2],
                                      in_=xt[62:66, 1, 1 + clo:LP + 2])
                    nc.sync.dma_start(out=xB[4:70, 1, clo:LP + 1],
                                      in_=xt[0:66, 1, 1 + clo:LP + 2])

        def fc1(s, blks):
            # fc1 + conv fused: 6 accumulating matmuls per psum block (3 taps
            # x 2 channel chunks, the aug-half read directly from xt with the
            # tap's column shift -- no shifted-copy build), then Gelu(psum +
            # dw_b) evacuates PSUM directly.  Block-major so the h0 pixel
            # blocks only depend on the h0 half of the prep chain.
            st = state[s]
            xt = st["xt"]
            if "t" not in st:
                st["t"] = tpool.tile([128, 3, L], BF16, name="t")
            t = st["t"]
            xB = st.get("xB")
            for blk in blks:
                cs = blk * NB
                for m in range(3):
                    pt_ = pf1.tile([128, NB], F32)
                    for tau in range(3):
                        nc.tensor.matmul(
                            pt_, lhsT=w1a[:, tau, m * 128:(m + 1) * 128],
                            rhs=xt[:, 0, cs + tau: cs + tau + NB],
                            start=(tau == 0), stop=False)
                    if USE_XB:
                        nc.tensor.matmul(
                            pt_, lhsT=w1b1[:, m * 128:(m + 1) * 128],
                            rhs=xB[:, 0, cs + 1: cs + 1 + NB],
                            start=False, stop=False)
                        nc.tensor.matmul(
                            pt_, lhsT=w1b2[0:70, m * 128:(m + 1) * 128],
                            rhs=xB[0:70, 1, cs + 1: cs + 1 + NB],
                            start=False, stop=True)
                    else:
                        for tau in range(3):
                            nc.tensor.matmul(
                                pt_,
                                lhsT=w1b[0:66, tau, m * 128:(m + 1) * 128],
                                rhs=xt[0:66, 1, cs + tau: cs + tau + NB],
                                start=False, stop=(tau == 2))
                    nc.scalar.activation(out=t[:, m, cs:cs + NB], in_=pt_,
                                         func=AF.Gelu, bias=gb[:, m:m + 1],
                                         scale=1.0)

        def fc2_g(s, g):
            # fc2 (stationary = gelu output -> pixel-major out); the
            # residual (x + fc2_b, preadded host-side) is added by the DVE
            # evacuation tensor_tensor reading PSUM + xb directly -- no
            # identity matmuls on the PE
            st = state[s]
            xb_sb, t = st["xb"], st["t"]
            y_sb = ypool.tile([128, 6, C], BF16)
            for jp in range(3):
                kp = 3 * g + jp
                py = pf2.tile([128, 2, C], F32)
                for j in range(2):
                    k = 2 * kp + j
                    for kc in range(3):
                        nc.tensor.matmul(
                            py[:, j, :],
                            lhsT=t[:, kc, k * PT:(k + 1) * PT],
                            rhs=w2[:, kc, :],
                            start=(kc == 0), stop=(kc == 2))
                nc.vector.tensor_tensor(out=y_sb[:, 2 * jp:2 * jp + 2, :],
                                        in0=py,
                                        in1=xb_sb[:, 6 * g + 2 * jp:
                                                  6 * g + 2 * jp + 2, :],
                                        op=AL.add)
            nc.sync.dma_start(out=y_d[:, s, 6 * g:6 * g + 6, :], in_=y_sb)

        def fc2_tail(s):
            # tail pixel tile (64 rows)
            st = state.pop(s)
            xb_sb, t = st["xb"], st["t"]
            py = pf2.tile([128, 2, C], F32)
            for kc in range(3):
                nc.tensor.matmul(py[0:TAIL, 0, :],
                                 lhsT=t[:, kc, (NT - 1) * PT: L],
                                 rhs=w2[:, kc, :],
                                 start=(kc == 0), stop=(kc == 2))
            y_sb = ypool.tile([128, 6, C], BF16, tag="ytail")
            nc.vector.tensor_tensor(out=y_sb[0:TAIL, 0, :],
                                    in0=py[0:TAIL, 0, :],
                                    in1=xb_sb[0:TAIL, NT - 1, :], op=AL.add)
            nc.sync.dma_start(out=y_d[0:TAIL, s, NT - 1, :],
                              in_=y_sb[0:TAIL, 0, :])

        # ---- emission schedule: prep stages interleave into the previous
        # sample's fc1/fc2 so every engine queue sees ops in dependency
        # order.  x loads go out before params (params aren't needed until
        # the first matmul ~25us in).
        warm_mms(36, 448)
        load(0)
        nc.scalar.dma_start(out=w1a, in_=w1a_d[:, :, :])
        if USE_XB:
            nc.scalar.dma_start(out=w1b1, in_=w1b1_d[:, :])
            nc.scalar.dma_start(out=w1b2, in_=w1b2_d[:, :])
        else:
            nc.scalar.dma_start(out=w1b, in_=w1b_d[:, :, :])
        nc.sync.dma_start(out=w2, in_=w2_d[:, :, :])
        nc.sync.dma_start(out=gb, in_=gb_d[:, :])
        # preload the gelu activation table
        warm = const.tile([128, 1], BF16)
        nc.scalar.activation(out=warm, in_=gb[:, 0:1], func=AF.Gelu,
                             bias=0.0, scale=1.0)
        stats(0, 0)
        stats(0, 1)
        warm_mms(16, 448)
        chain_a(0, 0)
        chain_a(0, 1)
        warm_mms(44, 448)
        chain_b(0, 0)
        chain_b(0, 1)
        if S > 1:
            load(1)
            stats(1, 0)
        for s in range(S):
            nxt = s + 1
            if nxt < S:
                fc1(s, range(0, 4))
                stats(nxt, 1)
                chain_a(nxt, 0)
                fc1(s, range(4, 6))
                chain_a(nxt, 1)
                fc1(s, range(6, NBLK))
                fc2_g(s, 0)
                chain_b(nxt, 0)
                fc2_g(s, 1)
                chain_b(nxt, 1)
                fc2_g(s, 2)
                fc2_g(s, 3)
                fc2_tail(s)
            else:
                fc1(s, range(0, NBLK))
                for g in range(4):
                    fc2_g(s, g)
                fc2_tail(s)
            if nxt + 1 < S:
                load(nxt + 1)
                stats(nxt + 1, 0)
    nc.finalize()
    return nc


def _get_nc(separate_stats=False):
    key = ("nc", separate_stats)
    if key not in _CACHE:
        _CACHE[key] = _build_nc(separate_stats)
    return _CACHE[key]


def _host_params(gamma, beta, fc1_w, fc1_b, dw_w, dw_b, fc2_w, fc2_b):
    bf = ml_dtypes.bfloat16
    w1g = (fc1_w * gamma[:, None]).astype(np.float32)          # [192, 384]
    s1g = w1g.sum(0)                                           # [384]
    b1aug = (beta @ fc1_w + fc1_b).astype(np.float32)          # [384]
    # mu aug row arrives doubled from the device stats combine -> halve here
    wfull = np.concatenate([w1g, -0.5 * s1g[None, :], b1aug[None, :]],
                           0)  # [194, D]
    k = dw_w[:, 0, :].astype(np.float32)                       # [384, 3]
    w1a = np.zeros((128, 3, D), dtype=bf)
    w1b = np.zeros((66, 3, D), dtype=bf)
    for tau in range(3):
        wt = wfull * k[:, tau][None, :]
        w1a[:, tau, :] = wt[0:128].astype(bf)
        w1b[:, tau, :] = wt[128:194].astype(bf)    # 66 aug-half rows per tap
    w2 = np.ascontiguousarray(
        fc2_w.reshape(3, 128, C).transpose(1, 0, 2)).astype(bf)  # [128,3,192]
    gb = np.ascontiguousarray(
        dw_b.reshape(3, 128).T).astype(np.float32)               # [128, 3]
    ident = np.eye(128, dtype=np.float32)
    params = dict(w1a=w1a, w2=w2, gb=gb, ident=ident)
    if USE_XB:
        params["w1b1"] = np.concatenate([w1b[:, 0, :], w1b[0:62, 1, :]], 0)
        params["w1b2"] = np.concatenate([w1b[62:66, 1, :], w1b[:, 2, :]], 0)
    else:
        params["w1b"] = w1b
    return params


def _host_xt(x_dev):
    """Channel-major bf16 copy of x with zero pad columns at 0 and L+1.
    xt0 [nb, 128, L+2] = channels 0..127; xt1 [nb, 66, L+2]: rows 0..63 =
    channels 128..191, row 64 = mu*rstd slot (runtime), row 65 = ones row
    (set here, zero at the pads)."""
    bf = ml_dtypes.bfloat16
    nb = x_dev.shape[0]
    arr = np.ascontiguousarray(
        x_dev.reshape(nb, L, C).transpose(0, 2, 1)).astype(bf)  # [nb, 192, L]
    xt0 = np.zeros((nb, 128, LP + 2), dtype=bf)
    xt1 = np.zeros((nb, 66, LP + 2), dtype=bf)
    xt0[:, :, 1:L + 1] = arr[:, 0:128]
    xt1[:, 0:64, 1:L + 1] = arr[:, 128:192]
    xt1[:, 65, 1:L + 1] = 1.0
    return xt0, xt1


def _selector_flags(x, gamma, beta, sel_w1, sel_b1, sel_w2, sel_b2):
    """Exact numpy replica of the reference direction selector. Only used
    when gamma is non-uniform (otherwise the scores tie and idx==0 always)."""
    xf = x.astype(np.float32)
    mu = xf.mean(-1, keepdims=True)
    var = ((xf - mu) ** 2).mean(-1, keepdims=True)
    xn = (xf - mu) / np.sqrt(var + EPS) * gamma + beta
    xg = xn.mean(-1)
    gh = np.abs(xg[:, :, 1:] - xg[:, :, :-1]).mean(axis=(1, 2))
    gv = np.abs(xg[:, 1:, :] - xg[:, :-1, :]).mean(axis=(1, 2))
    scores = np.stack([gh, gv, 0.8 * (gh + gv) * 0.5, np.abs(gh - gv)], 1)
    hdn = np.maximum(scores @ sel_w1 + sel_b1, 0.0)
    logits = hdn @ sel_w2 + sel_b2
    ex = np.exp(logits - logits.max(1, keepdims=True))
    probs = ex / ex.sum(1, keepdims=True)
    return probs.argmax(1) % 4 == 1


def build_in_maps(inputs):
    """Shared by kernel() and test harnesses: host preprocessing + sharding.
    Returns (in_maps, x, x_dev, flags)."""
    x = np.asarray(inputs["x"], dtype=np.float32)
    gamma = np.asarray(inputs["gamma"], np.float32)
    beta = np.asarray(inputs["beta"], np.float32)
    fc2_b = np.asarray(inputs["fc2_b"], np.float32)
    params = _host_params(
        gamma, beta,
        np.asarray(inputs["fc1_w"], np.float32),
        np.asarray(inputs["fc1_b"], np.float32),
        np.asarray(inputs["dw_w"], np.float32),
        np.asarray(inputs["dw_b"], np.float32),
        np.asarray(inputs["fc2_w"], np.float32),
        fc2_b,
    )

    # Routing: uniform gamma => gray image is constant => scores tie => idx 0
    # for every sample (see module docstring).  Otherwise compute the selector
    # on host and pre-transpose flagged samples (mathematically exact fixup).
    if np.ptp(gamma) == 0.0:
        flags = np.zeros(B, dtype=bool)
    else:
        flags = _selector_flags(
            x, gamma, beta,
            np.asarray(inputs["sel_w1"], np.float32),
            np.asarray(inputs["sel_b1"], np.float32),
            np.asarray(inputs["sel_w2"], np.float32),
            np.asarray(inputs["sel_b2"], np.float32))
    x_dev = x
    if flags.any():
        x_dev = x.copy()
        x_dev[flags] = np.swapaxes(x_dev[flags], 1, 2)

    separate_stats = bool(np.any(fc2_b != 0.0))
    xt0, xt1 = _host_xt(x_dev)
    xb = x_dev + fc2_b

    def _pmajor(v):
        # [S, H, W, C] -> [128, S, NT, C] partition-major pixel tiles
        bf = ml_dtypes.bfloat16
        v = v.reshape(-1, L, C)
        out = np.zeros((128, v.shape[0], NT, C), dtype=bf)
        out[:, :, 0:NT - 1, :] = (
            v[:, 0:(NT - 1) * PT].reshape(-1, NT - 1, PT, C)
            .transpose(2, 0, 1, 3))
        out[0:TAIL, :, NT - 1, :] = (
            v[:, (NT - 1) * PT:L].transpose(1, 0, 2))
        return out

    in_maps = []
    for i in range(NCORES):
        sl = slice(S * i, S * (i + 1))
        m = {"xb": _pmajor(xb[sl]),
             "xt0": xt0[sl],
             "xt1": xt1[sl]}
        if separate_stats:
            m["xstat"] = _pmajor(x_dev[sl])
        m.update(params)
        in_maps.append(m)
    return in_maps, x, x_dev, flags


def kernel(**inputs):
    from concourse.bass_utils import run_bass_kernel_spmd

    in_maps, x, x_dev, flags = build_in_maps(inputs)
    separate_stats = "xstat" in in_maps[0]
    nc = _get_nc(separate_stats)
    res = run_bass_kernel_spmd(nc, in_maps, list(range(NCORES)))
    # y arrives partition-major [128, S, NT, C]; un-transpose on host
    y = np.concatenate(
        [r["y"].astype(np.float32).transpose(1, 2, 0, 3)
             .reshape(S, NT * PT, C)[:, :L, :].reshape(S, H, W, C)
         for r in res.results], 0)
    if flags.any():
        # device computed x_dev + F(x_dev); reference wants x + F(x_dev)
        # (row-major unscan orientation is identical)
        y = x + (y - x_dev)
    return y.astype(np.float32)


# revision 76
# speedup vs baseline: 1.0116x; 1.0017x over previous
"""CASS block (LayerNorm + gradient-selected scan + fc1/dwconv/gelu/fc2 + residual)
on 8 TRN2 NeuronCores, pure data parallel over the batch.

Tensor-centric formulation: the depthwise 3-tap conv is folded into the fc1
matmul.  With rhs columns pre-scaled by the per-pixel LN rstd and two
augmented contraction rows (mu*rstd against -colsum(gamma*W1), and a ones row
against b1aug = beta@W1 + fc1_b, both zero at the conv pad columns), the fc1
PSUM accumulates, over 5 matmuls per block,

    psum[d, l] = sum_tau k_tau[d] * u[l+tau-1, d],   u = LN(x) @ W1 + b1,

i.e. the conv output directly.  The Scalar engine evacuates PSUM straight
through Gelu (bias = dw_b).  fc2 uses the gelu output as the stationary
operand so results come out pixel-major; the residual (+ x + fc2_b, preadded
host-side) is injected via an identity matmul into the same PSUM group.

Scheduling/efficiency notes (vs the first working version, 207us -> 177us):
 - warm-up matmuls at t=0 keep the PE HAM clock-gate at 2.4 GHz before the
   first real matmul and fill part of the prep window.
 - the aug-half taps are 3 direct K=66 matmuls against xt (6 matmuls per
   fc1 block).  The 5-matmul packed-xB variant is cheaper on paper but its
   2.1MB/sample of partition-shifted SBUF copies sit on the prep critical
   path and lose more to DMA-queue serialization than the 6th matmul costs
   (USE_XB toggles the packed variant back on).
 - fc2's residual is added by the DVE evacuation tensor_tensor (PSUM + xb)
   rather than identity matmuls: saves ~5us/sample of PE.
 - prep is split into per-half stages (stats -> transpose -> row-gather ->
   broadcast -> prescale) interleaved INTO the previous sample's fc1/fc2
   emission; fc1 emits block-major so pixel blocks 0..3 only gate on the
   h0 half; broadcast/prescale run in 3-4-tile column chunks so fc1 blocks
   unblock incrementally.
 - bn_stats (count, mean, count*var of even/odd element halves) is combined
   by ~13 strided vector ops per half instead of one bn_aggr per tile (each
   aggr pays a DVE pipeline-drain on its RAW dependency); the two halves'
   op chains are interleaved so the drain of one hides under the other.
   The mean lands doubled; the host halves the mu-aug weight row.  rstd is
   one division-free Newton step from z0 = 1.5 - 0.5v.
 - xb and y use partition-major HBM layouts (host transposes): the
   pixel-major rearrange moved 384-byte DMA packets which starve under
   packet-granularity round-robin against the large xt transfers.
 - xt half-1 ships only rows 0..65 (rows 66..127 are never read).
 - queue assignment: xb/y/aug/rstd-row on sync, xt on gpsimd (plus the
   partition_broadcasts, IRAM-warmed by a dummy broadcast at t=0), params +
   gelu evacuations on scalar, fc2 evacuation + stats on vector.
 - y is stored bf16 (host upcasts); halves the output DMA traffic.

The gradient selector: for uniform gamma the "gray" image mean_c(LN(x)) is a
constant, so grad_h = grad_v = 0, the MLP logits tie, softmax gives exactly
0.25 each in fp32, and argmax -> idx 0 for every sample: the 'v' (transpose)
branch is dead.  The device kernel therefore always scans row-major; a host
fallback handles non-uniform gamma by pre-transposing flagged samples."""

import numpy as np
import ml_dtypes

import concourse.mybir as mybir
import concourse.tile as tile
from concourse import bacc

B, H, W, C = 32, 56, 56, 192
D = 384                      # D_INNER
NCORES = 8
S = B // NCORES              # samples per core
L = H * W                    # 3136 pixels per sample
PT = 128                     # pixels per partition tile
NT = (L + PT - 1) // PT      # 25 pixel tiles (24 full + 64 tail)
TAIL = L - (NT - 1) * PT     # 64
NB = 448                     # fc1 N-block (one PSUM bank holds 448 f32)
NBLK = L // NB               # 7
LP = NT * PT                 # 3200: row-padded pixel count (25 full tiles)
EPS = 1e-5
USE_XB = False               # 5-matmul fc1 with pre-shifted xB pack
F32 = mybir.dt.float32
BF16 = mybir.dt.bfloat16
AL = mybir.AluOpType
AF = mybir.ActivationFunctionType

_CACHE = {}


def _build_nc(separate_stats: bool):
    nc = bacc.Bacc()
    # partition-major layouts: per-partition lines are contiguous so DMA
    # moves big packets (the pixel-major rearrange form moved 384-byte
    # packets and starved under packet-granularity queue round-robin)
    xb_d = nc.declare_dram_parameter("xb", [128, S, NT, C], BF16,
                                     isOutput=False)
    if separate_stats:
        xst_d = nc.declare_dram_parameter("xstat", [128, S, NT, C], BF16,
                                          isOutput=False)
    else:
        xst_d = xb_d
    xt0_d = nc.declare_dram_parameter("xt0", [S, 128, LP + 2], BF16,
                                      isOutput=False)
    xt1_d = nc.declare_dram_parameter("xt1", [S, 66, LP + 2], BF16,
                                      isOutput=False)
    w1a_d = nc.declare_dram_parameter("w1a", [128, 3, D], BF16, isOutput=False)
    if USE_XB:
        w1b1_d = nc.declare_dram_parameter("w1b1", [128, D], BF16,
                                           isOutput=False)
        w1b2_d = nc.declare_dram_parameter("w1b2", [70, D], BF16,
                                           isOutput=False)
    else:
        w1b_d = nc.declare_dram_parameter("w1b", [66, 3, D], BF16,
                                          isOutput=False)
    w2_d = nc.declare_dram_parameter("w2", [128, 3, C], BF16, isOutput=False)
    gb_d = nc.declare_dram_parameter("gb", [128, 3], F32, isOutput=False)
    id_d = nc.declare_dram_parameter("ident", [128, 128], F32, isOutput=False)
    y_d = nc.declare_dram_parameter("y", [128, S, NT, C], BF16, isOutput=True)

    with tile.TileContext(nc) as tc, \
         tc.tile_pool(name="const", bufs=1) as const, \
         tc.tile_pool(name="xb", bufs=4) as xbpool, \
         tc.tile_pool(name="xt", bufs=2) as xtpool, \
         tc.tile_pool(name="stat", bufs=4) as stat, \
         tc.tile_pool(name="rb", bufs=2) as rbpool, \
         tc.tile_pool(name="rr", bufs=2) as rrpool, \
         tc.tile_pool(name="xB", bufs=2) as xBpool, \
         tc.tile_pool(name="t", bufs=2) as tpool, \
         tc.tile_pool(name="y", bufs=3) as ypool, \
         tc.tile_pool(name="pf1", bufs=3, space="PSUM") as pf1, \
         tc.tile_pool(name="pf2", bufs=3, space="PSUM") as pf2, \
         tc.tile_pool(name="ptr", bufs=2, space="PSUM") as ptr:

        w1a = const.tile([128, 3, D], BF16)
        if USE_XB:
            w1b1 = const.tile([128, D], BF16)
            w1b2 = const.tile([70, D], BF16)
        else:
            w1b = const.tile([66, 3, D], BF16)
        w2 = const.tile([128, 3, C], BF16)
        gb = const.tile([128, 3], F32)
        ident = const.tile([128, 128], F32)
        wsrc = const.tile([128, 512], BF16)

        # wsrc memset first: warm-up matmuls depend only on it (no DMA)
        nc.vector.memset(wsrc, 0.0)
        nc.sync.dma_start(out=ident, in_=id_d[:, :])
        # warm the partition_broadcast ucode IRAM (~6us hidden first-use
        # cost) before the real broadcasts hit the critical path; input is
        # ident (already on chip) so it fires early
        bwarm = const.tile([128, 16], F32)
        nc.gpsimd.partition_broadcast(bwarm[:, :], ident[0:1, 0:16])

        def warm_mms(n, cols):
            # dummy matmuls: keep the PE HAM activity window busy so the
            # clock gate opens (and stays open) before real matmuls arrive
            wp = pf1.tile([128, NB], F32, tag="pt_")
            for _ in range(n):
                nc.tensor.matmul(wp[:, 0:cols], lhsT=wsrc[:, 0:128],
                                 rhs=wsrc[:, 0:cols], start=True, stop=True)

        state = {}

        def load(s):
            # pixel-major x (bf16): residual input + LN stats source
            xb_sb = xbpool.tile([128, NT, C], BF16)
            # split at the half boundary: the h0 stats (which gate the whole
            # prep chain) unblock without waiting for the h1 tiles' bytes
            nc.sync.dma_start(out=xb_sb[:, 0:16, :], in_=xb_d[:, s, 0:16, :])
            nc.sync.dma_start(out=xb_sb[:, 16:NT, :],
                              in_=xb_d[:, s, 16:NT, :])
            if separate_stats:
                xs_sb = xbpool.tile([128, NT, C], BF16, tag="xstat")
                nc.scalar.dma_start(out=xs_sb, in_=xst_d[:, s, :, :])
            else:
                xs_sb = xb_sb
            # channel-major bf16 x with pad cols + aug-row slots; half-1
            # rows 66..127 are never read, so only 66 rows ship from HBM.
            # First two samples ride the scalar queue (gpsimd is busy with
            # the broadcast-ucode warmup during the fill).
            xt = xtpool.tile([128, 2, LP + 2], BF16)
            nc.gpsimd.dma_start(out=xt[:, 0, :], in_=xt0_d[s, :, :])
            nc.gpsimd.dma_start(out=xt[0:66, 1, :], in_=xt1_d[s, :, :])
            st = {"xb": xb_sb, "xs": xs_sb, "xt": xt}
            state[s] = st

        def _combine_ops(s, h):
            # Closures for the per-half stats combine + Newton rsqrt.  Each
            # half's 13 ops form a serial RAW chain; the caller interleaves
            # the independent h0/h1 chains so the DVE pipeline-drain wait of
            # one hides under the other's execution.
            # bn_stats emits (count, mean, count*var) for even/odd element
            # halves; mean lands DOUBLED in the pack mu row (the host halves
            # the matching aug weight row); var = (cv_e+cv_o)/C +
            # (m_e-m_o)^2/4.  rstd via one division-free Newton step from
            # z0 = 1.5 - 0.5 v (per-pixel var concentrates near 1).
            # pack[:,h,0,k] = 2*mu_k -> 2*mu*rstd, pack[:,h,1,k] = rstd
            st = state[s]
            bns, pack, scr = st["bns"], st["pack"], st["scr"]
            tlo, thi = (0, 16) if h == 0 else (16, NT)
            nt = thi - tlo
            d_ = scr[:, h, 0, 0:nt]
            s_ = scr[:, h, 1, 0:nt]
            dd = scr[:, h, 2, 0:nt]
            me = bns[:, tlo:thi, 1:2].rearrange("p t o -> p (t o)")
            mo = bns[:, tlo:thi, 4:5].rearrange("p t o -> p (t o)")
            cve = bns[:, tlo:thi, 2:3].rearrange("p t o -> p (t o)")
            cvo = bns[:, tlo:thi, 5:6].rearrange("p t o -> p (t o)")
            tt, ts = nc.vector.tensor_tensor, nc.vector.tensor_scalar
            mu = pack[:, h, 0, 0:nt]
            rs = pack[:, h, 1, 0:nt]
            return [
                lambda: tt(out=mu, in0=me, in1=mo, op=AL.add),
                lambda: tt(out=d_, in0=me, in1=mo, op=AL.subtract),
                lambda: tt(out=s_, in0=cve, in1=cvo, op=AL.add),
                lambda: tt(out=dd, in0=d_, in1=d_, op=AL.mult),
                lambda: ts(out=s_, in0=s_, scalar1=1.0 / C, scalar2=EPS,
                           op0=AL.mult, op1=AL.add),
                lambda: ts(out=dd, in0=dd, scalar1=0.25, scalar2=None,
                           op0=AL.mult),
                lambda: tt(out=s_, in0=s_, in1=dd, op=AL.add),
                lambda: ts(out=d_, in0=s_, scalar1=-0.5, scalar2=1.5,
                           op0=AL.mult, op1=AL.add),
                lambda: tt(out=dd, in0=d_, in1=d_, op=AL.mult),
                lambda: tt(out=dd, in0=dd, in1=s_, op=AL.mult),
                lambda: ts(out=dd, in0=dd, scalar1=-0.5, scalar2=1.5,
                           op0=AL.mult, op1=AL.add),
                lambda: tt(out=rs, in0=d_, in1=dd, op=AL.mult),
                lambda: tt(out=mu, in0=mu, in1=rs, op=AL.mult),
            ]

        def stats(s, h):
            st = state[s]
            xs_sb = st["xs"]
            if h == 0:
                bns = stat.tile([128, NT, 6], F32)
                pack = stat.tile([128, 2, 2, 16], F32)
                scr = stat.tile([128, 2, 3, 16], F32, tag="scr")
                nc.vector.memset(pack, 0.0)
                st["bns"], st["pack"], st["scr"] = bns, pack, scr
                for k in range(0, 16):
                    nc.vector.bn_stats(out=bns[:, k:k + 1, :],
                                       in_=xs_sb[:, k:k + 1, :])
            else:
                bns = st["bns"]
                pre = [lambda: nc.vector.memset(bns[TAIL:128, NT - 1:NT, :],
                                                0.0)]
                for k in range(16, NT - 1):
                    pre.append(lambda k=k: nc.vector.bn_stats(
                        out=bns[:, k:k + 1, :], in_=xs_sb[:, k:k + 1, :]))
                pre.append(lambda: nc.vector.bn_stats(
                    out=bns[0:TAIL, NT - 1:NT, :],
                    in_=xs_sb[0:TAIL, NT - 1:NT, :]))
                if s == 0:
                    # fill path: h0 combine zips with the h1 bn_stats so the
                    # first transpose isn't gated on the h1 stats
                    from itertools import zip_longest
                    for op0, op1 in zip_longest(pre, _combine_ops(s, 0)):
                        if op0 is not None:
                            op0()
                        if op1 is not None:
                            op1()
                    for op1 in _combine_ops(s, 1):
                        op1()
                else:
                    for op0 in pre:
                        op0()
                    for op0, op1 in zip(_combine_ops(s, 0),
                                        _combine_ops(s, 1)):
                        op0()
                        op1()

        def chain_a(s, h):
            # PE-transpose stats to rows, extract rstd row + mu*rstd aug row,
            # broadcast rstd to all partitions
            st = state[s]
            pack, xt = st["pack"], st["xt"]
            nt = 16 if h == 0 else NT - 16
            clo, chi = (0, 16 * PT) if h == 0 else (16 * PT, NT * PT)
            if h == 0:
                rrow = rrpool.tile([1, LP], BF16)
                rstd_b = rbpool.tile([128, LP], BF16)
                st["rrow"], st["rstd_b"] = rrow, rstd_b
                if USE_XB:
                    st["xB"] = xBpool.tile([128, 2, LP + 2], BF16, name="xB")
            else:
                rrow, rstd_b = st["rrow"], st["rstd_b"]
            tpp = ptr.tile([32, 128], F32)
            nc.tensor.transpose(
                out=tpp[0:32, :],
                in_=pack[:, h, :, :].rearrange("p a b -> p (a b)"),
                identity=ident)
            packT = stat.tile([32, 128], BF16, tag=f"pT{h}")
            nc.vector.tensor_copy(out=packT, in_=tpp)
            # mu*rstd aug row (row 64 of half 1); ones row host-prepared
            nc.sync.dma_start(out=xt[64:65, 1, 1 + clo:1 + chi],
                              in_=packT[0:nt, :])
            # rstd row-gather + broadcast in column chunks so each
            # broadcast gates only on its own small DMA and downstream
            # prescale/fc1 blocks unblock incrementally
            tw = 4 if h == 0 else 3
            for j in range(0, nt, tw):
                jw = min(tw, nt - j)
                qlo, cw = clo + j * PT, jw * PT
                nc.sync.dma_start(out=rrow[0:1, qlo:qlo + cw],
                                  in_=packT[16 + j:16 + j + jw, :])
                nc.gpsimd.partition_broadcast(rstd_b[:, qlo:qlo + cw],
                                              rrow[0:1, qlo:qlo + cw])

        def chain_b(s, h):
            # prescale xt by rstd in place (aug rows 64/65 excluded), in
            # column chunks matching the broadcast chunks
            st = state[s]
            xt, rstd_b = st["xt"], st["rstd_b"]
            clo, chi = (0, 16 * PT) if h == 0 else (16 * PT, NT * PT)
            nt = (chi - clo) // PT
            tw = 4 if h == 0 else 3
            for j in range(0, nt, tw):
                jw = min(tw, nt - j)
                qlo, cw = clo + j * PT, jw * PT
                nc.vector.tensor_tensor(out=xt[:, 0, 1 + qlo:1 + qlo + cw],
                                        in0=xt[:, 0, 1 + qlo:1 + qlo + cw],
                                        in1=rstd_b[:, qlo:qlo + cw],
                                        op=AL.mult)
                nc.vector.tensor_tensor(out=xt[0:64, 1, 1 + qlo:1 + qlo + cw],
                                        in0=xt[0:64, 1, 1 + qlo:1 + qlo + cw],
                                        in1=rstd_b[0:64, qlo:qlo + cw],
                                        op=AL.mult)
            if USE_XB:
                xB = st["xB"]
                if h == 0:
                    nc.sync.dma_start(out=xB[0:66, 0, 1:2 + chi],
                                      in_=xt[0:66, 1, 0:1 + chi])
                    nc.sync.dma_start(out=xB[66:128, 0, 0:1 + chi],
                                      in_=xt[0:62, 1, 0:1 + chi])
                    nc.sync.dma_start(out=xB[0:4, 1, 0:1 + chi],
                                      in_=xt[62:66, 1, 0:1 + chi])
                    nc.sync.dma_start(out=xB[4:70, 1, 0:chi],
                                      in_=xt[0:66, 1, 1:1 + chi])
                else:
                    nc.sync.dma_start(out=xB[0:66, 0, 2 + clo:LP + 2],
                                      in_=xt[0:66, 1, 1 + clo:LP + 1])
                    nc.sync.dma_start(out=xB[66:128, 0, 1 + clo:LP + 2],
                                      in_=xt[0:62, 1, 1 + clo:LP + 2])
                    nc.sync.dma_start(out=xB[0:4, 1, 1 + clo:LP + 2],
                                      in_=xt[62:66, 1, 1 + clo:LP + 2])
                    nc.sync.dma_start(out=xB[4:70, 1, clo:LP + 1],
                                      in_=xt[0:66, 1, 1 + clo:LP + 2])

        def fc1(s, blks):
            # fc1 + conv fused: 6 accumulating matmuls per psum block (3 taps
            # x 2 channel chunks, the aug-half read directly from xt with the
            # tap's column shift -- no shifted-copy build), then Gelu(psum +
            # dw_b) evacuates PSUM directly.  Block-major so the h0 pixel
            # blocks only depend on the h0 half of the prep chain.
            st = state[s]
            xt = st["xt"]
            if "t" not in st:
                st["t"] = tpool.tile([128, 3, L], BF16, name="t")
            t = st["t"]
            xB = st.get("xB")
            for blk in blks:
                cs = blk * NB
                for m in range(3):
                    pt_ = pf1.tile([128, NB], F32)
                    for tau in range(3):
                        nc.tensor.matmul(
                            pt_, lhsT=w1a[:, tau, m * 128:(m + 1) * 128],
                            rhs=xt[:, 0, cs + tau: cs + tau + NB],
                            start=(tau == 0), stop=False)
                    if USE_XB:
                        nc.tensor.matmul(
                            pt_, lhsT=w1b1[:, m * 128:(m + 1) * 128],
                            rhs=xB[:, 0, cs + 1: cs + 1 + NB],
                            start=False, stop=False)
                        nc.tensor.matmul(
                            pt_, lhsT=w1b2[0:70, m * 128:(m + 1) * 128],
                            rhs=xB[0:70, 1, cs + 1: cs + 1 + NB],
                            start=False, stop=True)
                    else:
                        for tau in range(3):
                            nc.tensor.matmul(
                                pt_,
                                lhsT=w1b[0:66, tau, m * 128:(m + 1) * 128],
                                rhs=xt[0:66, 1, cs + tau: cs + tau + NB],
                                start=False, stop=(tau == 2))
                    nc.scalar.activation(out=t[:, m, cs:cs + NB], in_=pt_,
                                         func=AF.Gelu, bias=gb[:, m:m + 1],
                                         scale=1.0)

        def fc2_g(s, g):
            # fc2 (stationary = gelu output -> pixel-major out); the
            # residual (x + fc2_b, preadded host-side) is added by the DVE
            # evacuation tensor_tensor reading PSUM + xb directly -- no
            # identity matmuls on the PE
            st = state[s]
            xb_sb, t = st["xb"], st["t"]
            y_sb = ypool.tile([128, 6, C], BF16)
            for jp in range(3):
                kp = 3 * g + jp
                py = pf2.tile([128, 2, C], F32)
                for j in range(2):
                    k = 2 * kp + j
                    for kc in range(3):
                        nc.tensor.matmul(
                            py[:, j, :],
                            lhsT=t[:, kc, k * PT:(k + 1) * PT],
                            rhs=w2[:, kc, :],
                            start=(kc == 0), stop=(kc == 2))
                nc.vector.tensor_tensor(out=y_sb[:, 2 * jp:2 * jp + 2, :],
                                        in0=py,
                                        in1=xb_sb[:, 6 * g + 2 * jp:
                                                  6 * g + 2 * jp + 2, :],
                                        op=AL.add)
            nc.sync.dma_start(out=y_d[:, s, 6 * g:6 * g + 6, :], in_=y_sb)

        def fc2_tail(s):
            # tail pixel tile (64 rows)
            st = state.pop(s)
            xb_sb, t = st["xb"], st["t"]
            py = pf2.tile([128, 2, C], F32)
            for kc in range(3):
                nc.tensor.matmul(py[0:TAIL, 0, :],
                                 lhsT=t[:, kc, (NT - 1) * PT: L],
                                 rhs=w2[:, kc, :],
                                 start=(kc == 0), stop=(kc == 2))
            y_sb = ypool.tile([128, 6, C], BF16, tag="ytail")
            nc.vector.tensor_tensor(out=y_sb[0:TAIL, 0, :],
                                    in0=py[0:TAIL, 0, :],
                                    in1=xb_sb[0:TAIL, NT - 1, :], op=AL.add)
            nc.sync.dma_start(out=y_d[0:TAIL, s, NT - 1, :],
                              in_=y_sb[0:TAIL, 0, :])

        # ---- emission schedule: prep stages interleave into the previous
        # sample's fc1/fc2 so every engine queue sees ops in dependency
        # order.  x loads go out before params (params aren't needed until
        # the first matmul ~25us in).
        warm_mms(36, 448)
        load(0)
        nc.scalar.dma_start(out=w1a, in_=w1a_d[:, :, :])
        if USE_XB:
            nc.scalar.dma_start(out=w1b1, in_=w1b1_d[:, :])
            nc.scalar.dma_start(out=w1b2, in_=w1b2_d[:, :])
        else:
            nc.scalar.dma_start(out=w1b, in_=w1b_d[:, :, :])
        nc.sync.dma_start(out=w2, in_=w2_d[:, :, :])
        nc.sync.dma_start(out=gb, in_=gb_d[:, :])
        # preload the gelu activation table
        warm = const.tile([128, 1], BF16)
        nc.scalar.activation(out=warm, in_=gb[:, 0:1], func=AF.Gelu,
                             bias=0.0, scale=1.0)
        stats(0, 0)
        stats(0, 1)
        warm_mms(16, 448)
        chain_a(0, 0)
        chain_a(0, 1)
        warm_mms(52, 448)
        chain_b(0, 0)
        chain_b(0, 1)
        if S > 1:
            load(1)
            stats(1, 0)
        for s in range(S):
            nxt = s + 1
            if nxt < S:
                fc1(s, range(0, 4))
                stats(nxt, 1)
                chain_a(nxt, 0)
                fc1(s, range(4, 6))
                chain_a(nxt, 1)
                fc1(s, range(6, NBLK))
                fc2_g(s, 0)
                chain_b(nxt, 0)
                fc2_g(s, 1)
                chain_b(nxt, 1)
                fc2_g(s, 2)
                fc2_g(s, 3)
                fc2_tail(s)
            else:
                fc1(s, range(0, NBLK))
                for g in range(4):
                    fc2_g(s, g)
                fc2_tail(s)
            if nxt + 1 < S:
                load(nxt + 1)
                stats(nxt + 1, 0)
    nc.finalize()
    return nc


def _get_nc(separate_stats=False):
    key = ("nc", separate_stats)
    if key not in _CACHE:
        _CACHE[key] = _build_nc(separate_stats)
    return _CACHE[key]


def _host_params(gamma, beta, fc1_w, fc1_b, dw_w, dw_b, fc2_w, fc2_b):
    bf = ml_dtypes.bfloat16
    w1g = (fc1_w * gamma[:, None]).astype(np.float32)          # [192, 384]
    s1g = w1g.sum(0)                                           # [384]
    b1aug = (beta @ fc1_w + fc1_b).astype(np.float32)          # [384]
    # mu aug row arrives doubled from the device stats combine -> halve here
    wfull = np.concatenate([w1g, -0.5 * s1g[None, :], b1aug[None, :]],
                           0)  # [194, D]
    k = dw_w[:, 0, :].astype(np.float32)                       # [384, 3]
    w1a = np.zeros((128, 3, D), dtype=bf)
    w1b = np.zeros((66, 3, D), dtype=bf)
    for tau in range(3):
        wt = wfull * k[:, tau][None, :]
        w1a[:, tau, :] = wt[0:128].astype(bf)
        w1b[:, tau, :] = wt[128:194].astype(bf)    # 66 aug-half rows per tap
    w2 = np.ascontiguousarray(
        fc2_w.reshape(3, 128, C).transpose(1, 0, 2)).astype(bf)  # [128,3,192]
    gb = np.ascontiguousarray(
        dw_b.reshape(3, 128).T).astype(np.float32)               # [128, 3]
    ident = np.eye(128, dtype=np.float32)
    params = dict(w1a=w1a, w2=w2, gb=gb, ident=ident)
    if USE_XB:
        params["w1b1"] = np.concatenate([w1b[:, 0, :], w1b[0:62, 1, :]], 0)
        params["w1b2"] = np.concatenate([w1b[62:66, 1, :], w1b[:, 2, :]], 0)
    else:
        params["w1b"] = w1b
    return params


def _host_xt(x_dev):
    """Channel-major bf16 copy of x with zero pad columns at 0 and L+1.
    xt0 [nb, 128, L+2] = channels 0..127; xt1 [nb, 66, L+2]: rows 0..63 =
    channels 128..191, row 64 = mu*rstd slot (runtime), row 65 = ones row
    (set here, zero at the pads)."""
    bf = ml_dtypes.bfloat16
    nb = x_dev.shape[0]
    arr = np.ascontiguousarray(
        x_dev.reshape(nb, L, C).transpose(0, 2, 1)).astype(bf)  # [nb, 192, L]
    xt0 = np.zeros((nb, 128, LP + 2), dtype=bf)
    xt1 = np.zeros((nb, 66, LP + 2), dtype=bf)
    xt0[:, :, 1:L + 1] = arr[:, 0:128]
    xt1[:, 0:64, 1:L + 1] = arr[:, 128:192]
    xt1[:, 65, 1:L + 1] = 1.0
    return xt0, xt1


def _selector_flags(x, gamma, beta, sel_w1, sel_b1, sel_w2, sel_b2):
    """Exact numpy replica of the reference direction selector. Only used
    when gamma is non-uniform (otherwise the scores tie and idx==0 always)."""
    xf = x.astype(np.float32)
    mu = xf.mean(-1, keepdims=True)
    var = ((xf - mu) ** 2).mean(-1, keepdims=True)
    xn = (xf - mu) / np.sqrt(var + EPS) * gamma + beta
    xg = xn.mean(-1)
    gh = np.abs(xg[:, :, 1:] - xg[:, :, :-1]).mean(axis=(1, 2))
    gv = np.abs(xg[:, 1:, :] - xg[:, :-1, :]).mean(axis=(1, 2))
    scores = np.stack([gh, gv, 0.8 * (gh + gv) * 0.5, np.abs(gh - gv)], 1)
    hdn = np.maximum(scores @ sel_w1 + sel_b1, 0.0)
    logits = hdn @ sel_w2 + sel_b2
    ex = np.exp(logits - logits.max(1, keepdims=True))
    probs = ex / ex.sum(1, keepdims=True)
    return probs.argmax(1) % 4 == 1


def build_in_maps(inputs):
    """Shared by kernel() and test harnesses: host preprocessing + sharding.
    Returns (in_maps, x, x_dev, flags)."""
    x = np.asarray(inputs["x"], dtype=np.float32)
    gamma = np.asarray(inputs["gamma"], np.float32)
    beta = np.asarray(inputs["beta"], np.float32)
    fc2_b = np.asarray(inputs["fc2_b"], np.float32)
    params = _host_params(
        gamma, beta,
        np.asarray(inputs["fc1_w"], np.float32),
        np.asarray(inputs["fc1_b"], np.float32),
        np.asarray(inputs["dw_w"], np.float32),
        np.asarray(inputs["dw_b"], np.float32),
        np.asarray(inputs["fc2_w"], np.float32),
        fc2_b,
    )

    # Routing: uniform gamma => gray image is constant => scores tie => idx 0
    # for every sample (see module docstring).  Otherwise compute the selector
    # on host and pre-transpose flagged samples (mathematically exact fixup).
    if np.ptp(gamma) == 0.0:
        flags = np.zeros(B, dtype=bool)
    else:
        flags = _selector_flags(
            x, gamma, beta,
            np.asarray(inputs["sel_w1"], np.float32),
            np.asarray(inputs["sel_b1"], np.float32),
            np.asarray(inputs["sel_w2"], np.float32),
            np.asarray(inputs["sel_b2"], np.float32))
    x_dev = x
    if flags.any():
        x_dev = x.copy()
        x_dev[flags] = np.swapaxes(x_dev[flags], 1, 2)

    separate_stats = bool(np.any(fc2_b != 0.0))
    xt0, xt1 = _host_xt(x_dev)
    xb = x_dev + fc2_b

    def _pmajor(v):
        # [S, H, W, C] -> [128, S, NT, C] partition-major pixel tiles
        bf = ml_dtypes.bfloat16
        v = v.reshape(-1, L, C)
        out = np.zeros((128, v.shape[0], NT, C), dtype=bf)
        out[:, :, 0:NT - 1, :] = (
            v[:, 0:(NT - 1) * PT].reshape(-1, NT - 1, PT, C)
            .transpose(2, 0, 1, 3))
        out[0:TAIL, :, NT - 1, :] = (
            v[:, (NT - 1) * PT:L].transpose(1, 0, 2))
        return out

    in_maps = []
    for i in range(NCORES):
        sl = slice(S * i, S * (i + 1))
        m = {"xb": _pmajor(xb[sl]),
             "xt0": xt0[sl],
             "xt1": xt1[sl]}
        if separate_stats:
            m["xstat"] = _pmajor(x_dev[sl])
        m.update(params)
        in_maps.append(m)
    return in_maps, x, x_dev, flags


def kernel(**inputs):
    from concourse.bass_utils import run_bass_kernel_spmd

    in_maps, x, x_dev, flags = build_in_maps(inputs)
    separate_stats = "xstat" in in_maps[0]
    nc = _get_nc(separate_stats)
    res = run_bass_kernel_spmd(nc, in_maps, list(range(NCORES)))
    # y arrives partition-major [128, S, NT, C]; un-transpose on host
    y = np.concatenate(
        [r["y"].astype(np.float32).transpose(1, 2, 0, 3)
             .reshape(S, NT * PT, C)[:, :L, :].reshape(S, H, W, C)
         for r in res.results], 0)
    if flags.any():
        # device computed x_dev + F(x_dev); reference wants x + F(x_dev)
        # (row-major unscan orientation is identical)
        y = x + (y - x_dev)
    return y.astype(np.float32)
